# revision 5
# baseline (speedup 1.0000x reference)
"""Supervised contrastive loss kernel for Trainium2 (8 NeuronCores, Bass/Tile).

Row-parallel sharding with host-side input staging and symmetric halving:
core c owns rows [c*1024, (c+1)*1024).  The host L2-normalizes rows (scale
folded with 1/sqrt(temperature)), transposes to gT [D, N], quantizes to
fp8e4m3 (x16 scaling for range), and hands each core 5 of the 8
1024-column blocks in circulant order (own block first, then the next 4
mod 8) -- exploiting sim-matrix symmetry: block pair (a, b) is computed
once, and its exp column-sums are routed back to the owner of rows b.

Per core (d = block index 0..4):
  - sim chunk [128, 1024] per (d, m-tile) via fp8 DoubleRow matmuls
    (2 fp8 weights per PE cell => K=256 per instruction, fp32 PSUM accum),
  - d=0: additive -1e9 diagonal-window mask (SPMD-uniform thanks to the
    rotation), exp in place on PSUM with fused row-sum accumulation,
  - d=1..3: exp written to SBUF bf16 (+ fused row-sum); ones-vector
    matmuls produce exp column-sums, accumulated into a [1, 3072] buffer,
  - d=4: exp row-sums only (the transpose pair computes its own row-sums),
  - P^T = C^T @ G (class-sum matrix C stationary, fp8) for positive sums.
Outputs per core: den [128, 8], colsums [1, 3072], P^T [32, 1024].

Host combines: den_total[row] = own row-sums + 3 routed column-sums;
pos_i = P[i, type_i]/16 - 1/T; then the standard log-form reduction.
"""

import numpy as np
import ml_dtypes

import concourse.bass as bass
import concourse.bacc as bacc
import concourse.mybir as mybir
from concourse import tile
from concourse.bass_utils import run_bass_kernel_spmd

N, D, NT, NC = 8192, 1024, 32, 8
R = N // NC          # rows per core
RT = R // 128        # row tiles per core (m-tiles)
KT = D // 128        # contraction chunks
NB = 5               # column blocks computed per core (symmetric halving)
NCS = 3              # blocks with column-sum routing (d = 1..3)
T = 0.07
EPS = 1e-10
NEG = -1.0e9
SC = 16.0            # fp8 pre-scale per operand (PSUM carries SC^2 * sim)
WWIN = 1920          # width of the shifted diagonal-mask input

F32 = mybir.dt.float32
BF16 = mybir.dt.bfloat16
F8 = mybir.dt.float8e4
BF16_NP = ml_dtypes.bfloat16
F8_NP = ml_dtypes.float8_e4m3


def build_program():
    nc = bacc.Bacc(None, target_bir_lowering=False, debug=False)
    gx = nc.dram_tensor("gx", [128, NB * KT * 1024], F8, kind="ExternalInput")
    cc8 = nc.dram_tensor("cc8", [128, KT * NT], F8, kind="ExternalInput")
    wm = nc.dram_tensor("wm", [128, WWIN], BF16, kind="ExternalInput")
    one1 = nc.dram_tensor("one1", [128, 1], BF16, kind="ExternalInput")
    deno = nc.dram_tensor("den_o", [128, RT], F32, kind="ExternalOutput")
    cso = nc.dram_tensor("cs_o", [1, NCS * 1024], F32, kind="ExternalOutput")
    poso = nc.dram_tensor("pos_o", [NT, R], F32, kind="ExternalOutput")

    AX = mybir.AxisListType.X
    AF = mybir.ActivationFunctionType
    DR = mybir.MatmulPerfMode.DoubleRow

    with tile.TileContext(nc) as tc:
        with (
            tc.tile_pool(name="big", bufs=1) as big,
            tc.tile_pool(name="work", bufs=3) as work,
            tc.tile_pool(name="stats", bufs=1) as stats,
            tc.tile_pool(name="psum", bufs=2, space="PSUM") as psum,
            tc.tile_pool(name="pcs", bufs=2, space="PSUM") as pcs,
            tc.tile_pool(name="ppt", bufs=2, space="PSUM") as ppt,
        ):
            gsb = [
                big.tile([128, KT, 1024], F8, tag=f"g{d}", name=f"g{d}")
                for d in range(NB)
            ]
            csb = big.tile([128, KT * NT], F8, tag="cc")
            wsb = big.tile([128, WWIN], BF16, tag="wm")
            osb = big.tile([128, 1], BF16, tag="one")
            acc = stats.tile([128, RT, NB], F32, tag="acc")
            den = stats.tile([128, RT], F32, tag="den")
            cs = stats.tile([1, NCS * 1024], F32, tag="cs")
            post = stats.tile([NT, R], F32, tag="post")

            nc.gpsimd.dma_start(csb[:, :], cc8[:, :])
            nc.gpsimd.dma_start(wsb[:, :], wm[:, :])
            nc.gpsimd.dma_start(osb[:, :], one1[:, :])
            for d in range(NB):
                nc.gpsimd.dma_start(
                    gsb[d][:, :, :], gx[:, d * KT * 1024 : (d + 1) * KT * 1024]
                )
            # zero the column-sum accumulator
            nc.vector.memset(cs[:, :], 0.0)

            for d in range(NB):
                for mt in range(RT):
                    ms = slice(mt * 128, (mt + 1) * 128)
                    sp = psum.tile([128, 1024], F32, tag="sim")
                    for h in range(2):
                        for kp in range(KT // 2):
                            nc.tensor.matmul(
                                sp[:, h * 512 : (h + 1) * 512],
                                gsb[0][:, 2 * kp : 2 * kp + 2, ms],
                                gsb[d][:, 2 * kp : 2 * kp + 2, h * 512 : (h + 1) * 512],
                                start=(kp == 0),
                                stop=(kp == KT // 2 - 1),
                                perf_mode=DR,
                            )
                    if d == 0:
                        woff = 896 - 128 * mt
                        nc.vector.tensor_add(
                            sp[:, :], sp[:, :], wsb[:, woff : woff + 1024]
                        )
                    if 1 <= d <= NCS:
                        ex = work.tile([128, 1024], BF16, tag="ex")
                        nc.scalar.activation(
                            ex[:, :], sp[:, :], AF.Exp,
                            scale=1.0 / (SC * SC),
                            accum_out=acc[:, mt, d : d + 1],
                        )
                        for h in range(2):
                            cp = pcs.tile([1, 512], F32, tag="cp")
                            nc.tensor.matmul(
                                cp[:, :],
                                osb[:, :],
                                ex[:, h * 512 : (h + 1) * 512],
                                start=True,
                                stop=True,
                            )
                            off = (d - 1) * 1024 + h * 512
                            nc.vector.tensor_add(
                                cs[:, off : off + 512], cs[:, off : off + 512], cp[:, :]
                            )
                    else:
                        nc.scalar.activation(
                            sp[:, :], sp[:, :], AF.Exp,
                            scale=1.0 / (SC * SC),
                            accum_out=acc[:, mt, d : d + 1],
                        )

            for mt in range(RT):
                nc.vector.reduce_sum(den[:, mt : mt + 1], acc[:, mt, :], axis=AX)

            # P^T = C^T @ G over own rows (C stationary, fp8)
            for h in range(2):
                pt = ppt.tile([NT, 512], F32, tag="pt")
                for kt in range(KT):
                    nc.tensor.matmul(
                        pt[:, :],
                        csb[:, kt * NT : (kt + 1) * NT],
                        gsb[0][:, kt, h * 512 : (h + 1) * 512],
                        start=(kt == 0),
                        stop=(kt == KT - 1),
                    )
                nc.vector.tensor_copy(post[:, h * 512 : (h + 1) * 512], pt[:, :])

            nc.gpsimd.dma_start(deno[:, :], den[:, :])
            nc.gpsimd.dma_start(cso[:, :], cs[:, :])
            nc.gpsimd.dma_start(poso[:, :], post[:, :])

    nc.compile()
    return nc


_NC_CACHE = None
_last_in_maps = None


def _get_program():
    global _NC_CACHE
    if _NC_CACHE is None:
        _NC_CACHE = build_program()
    return _NC_CACHE


def _build_in_maps(f, t):
    """f: [N, D] float32 features, t: [N] int64 types -> per-core input dicts."""
    nrm = np.maximum(np.sqrt((f.astype(np.float64) ** 2).sum(axis=1)), 1e-12)
    g = (f / nrm[:, None].astype(np.float32)) * np.float32(1.0 / np.sqrt(T))
    g8 = (g * np.float32(SC)).astype(F8_NP)            # [N, D] fp8, x16

    # per-block [128, KT*1024]: B[b][p, kt*1024 + j] = g8.T[kt*128+p, b*1024+j]
    gT = np.ascontiguousarray(g8.T)                     # [D, N]
    Gb = gT.reshape(KT, 128, NC, 1024)                  # [kt, p, b, j]
    blocks = [
        np.ascontiguousarray(Gb[:, :, b, :].transpose(1, 0, 2).reshape(128, KT * 1024))
        for b in range(NC)
    ]

    OH = (t[:, None] == np.arange(NT)[None, :]).astype(np.float32)
    C = g.T @ OH                                        # [D, NT] fp32 (class sums)
    c8 = np.ascontiguousarray(
        C.reshape(KT, 128, NT).transpose(1, 0, 2).reshape(128, KT * NT)
    ).astype(F8_NP)

    wmask = np.zeros((128, WWIN), BF16_NP)
    wmask[np.arange(128), 896 + np.arange(128)] = BF16_NP(NEG)
    one1 = np.ones((128, 1), BF16_NP)

    in_maps = []
    for c in range(NC):
        in_maps.append(
            {
                "gx": np.concatenate(
                    [blocks[(c + d) % NC] for d in range(NB)], axis=1
                ),
                "cc8": c8,
                "wm": wmask,
                "one1": one1,
            }
        )
    return in_maps


def kernel(features, element_types):
    f = np.ascontiguousarray(np.asarray(features), dtype=np.float32)
    t = np.asarray(element_types).astype(np.int64)
    assert f.shape == (N, D) and t.shape == (N,)

    hist = np.bincount(t, minlength=NT)
    cnt = hist[t] - 1
    valid = cnt > 0
    in_maps = _build_in_maps(f, t)
    global _last_in_maps
    _last_in_maps = in_maps

    nc = _get_program()
    res = run_bass_kernel_spmd(nc, in_maps, list(range(NC))).results

    # den_total[i] = own row-sums (blocks d=0..4) + column-sums routed from
    # the cores that computed blocks whose columns are rows of core b.
    den_total = np.zeros(N, dtype=np.float64)
    for c in range(NC):
        den_total[c * R : (c + 1) * R] += np.asarray(
            res[c]["den_o"], dtype=np.float64
        ).T.ravel()
        cs = np.asarray(res[c]["cs_o"], dtype=np.float64).ravel()
        for d in range(1, NCS + 1):
            b = (c + d) % NC
            den_total[b * R : (b + 1) * R] += cs[(d - 1) * 1024 : d * 1024]

    total = 0.0
    vc = int(valid.sum())
    for c in range(NC):
        PT = np.asarray(res[c]["pos_o"], dtype=np.float64) / SC   # [NT, R]
        rows = slice(c * R, (c + 1) * R)
        pos_sum = PT[t[rows], np.arange(R)] - 1.0 / T
        pm = pos_sum / np.maximum(cnt[rows], 1)
        loss = -np.log(np.exp(pm) / den_total[rows] + EPS)
        total += float((loss * valid[rows]).sum())
    out = total / vc if vc > 0 else 0.0
    return np.float32(out)


# revision 17
# speedup vs baseline: 1.0382x; 1.0382x over previous
"""Supervised contrastive loss kernel for Trainium2 (8 NeuronCores, Bass/Tile).

Row-parallel sharding with host-side input staging and symmetric halving:
core c owns rows [c*1024, (c+1)*1024).  The host L2-normalizes rows (scale
folded with 1/sqrt(temperature)), transposes to gT [D, N], quantizes to
fp8e4m3 (x16 scaling for range), and hands each core 5 of the 8
1024-column blocks in circulant order (own block first, then the next 4
mod 8) -- exploiting sim-matrix symmetry: block pair (a, b) is computed
once, and its exp column-sums are routed back to the owner of rows b.

Per core (d = block index 0..4):
  - sim chunk [128, 1024] per (d, m-tile) via fp8 DoubleRow matmuls
    (2 fp8 weights per PE cell => K=256 per instruction, fp32 PSUM accum),
  - d=0: additive -1e9 diagonal-window mask (SPMD-uniform thanks to the
    rotation), exp in place on PSUM with fused row-sum accumulation,
  - d=1..3: exp written to SBUF bf16 (+ fused row-sum); ones-vector
    matmuls produce exp column-sums, accumulated into a [1, 3072] buffer,
  - d=4: exp row-sums only (the transpose pair computes its own row-sums),
  - P^T = C^T @ G (class-sum matrix C stationary, fp8) for positive sums.
Outputs per core: den [128, 8], colsums [1, 3072], P^T [32, 1024].

Host combines: den_total[row] = own row-sums + 3 routed column-sums;
pos_i = P[i, type_i]/16 - 1/T; then the standard log-form reduction.
"""

import numpy as np
import ml_dtypes

import concourse.bass as bass
import concourse.bacc as bacc
import concourse.mybir as mybir
from concourse import tile
from concourse.bass_utils import run_bass_kernel_spmd

N, D, NT, NC = 8192, 1024, 32, 8
R = N // NC          # rows per core
RT = R // 128        # row tiles per core (m-tiles)
KT = D // 128        # contraction chunks
NB = 5               # column blocks computed per core (symmetric halving)
NCS = 3              # blocks with column-sum routing (d = 1..3)
T = 0.07
EPS = 1e-10
NEG = -1.0e9
SC = 16.0            # fp8 pre-scale per operand (PSUM carries SC^2 * sim)
WWIN = 1920          # width of the shifted diagonal-mask input

F32 = mybir.dt.float32
BF16 = mybir.dt.bfloat16
F8 = mybir.dt.float8e4
BF16_NP = ml_dtypes.bfloat16
F8_NP = ml_dtypes.float8_e4m3


def build_program():
    nc = bacc.Bacc(None, target_bir_lowering=False, debug=False)
    gx = nc.dram_tensor("gx", [128, NB * KT * 1024], F8, kind="ExternalInput")
    cc8 = nc.dram_tensor("cc8", [128, KT * NT], F8, kind="ExternalInput")
    wm = nc.dram_tensor("wm", [128, WWIN], BF16, kind="ExternalInput")
    one1 = nc.dram_tensor("one1", [128, 1], BF16, kind="ExternalInput")
    deno = nc.dram_tensor("den_o", [128, RT], F32, kind="ExternalOutput")
    cso = nc.dram_tensor("cs_o", [1, NCS * 1024], F32, kind="ExternalOutput")
    poso = nc.dram_tensor("pos_o", [NT, R], F32, kind="ExternalOutput")

    AX = mybir.AxisListType.X
    AF = mybir.ActivationFunctionType
    DR = mybir.MatmulPerfMode.DoubleRow

    with tile.TileContext(nc) as tc:
        with (
            tc.tile_pool(name="big", bufs=1) as big,
            tc.tile_pool(name="work", bufs=3) as work,
            tc.tile_pool(name="stats", bufs=1) as stats,
            tc.tile_pool(name="psum", bufs=4, space="PSUM") as psum,
        ):
            gsb = [
                big.tile([128, KT, 1024], F8, tag=f"g{d}", name=f"g{d}")
                for d in range(NB)
            ]
            csb = big.tile([128, KT * NT], F8, tag="cc")
            wsb = big.tile([128, WWIN], BF16, tag="wm")
            osb = big.tile([128, 1], BF16, tag="one")
            acc = stats.tile([128, RT, NB], F32, tag="acc")
            den = stats.tile([128, RT], F32, tag="den")
            cs = stats.tile([1, NCS * 1024], F32, tag="cs")
            post = stats.tile([NT, R], F32, tag="post")

            nc.gpsimd.dma_start(csb[:, :], cc8[:, :])
            nc.gpsimd.dma_start(wsb[:, :], wm[:, :])
            nc.gpsimd.dma_start(osb[:, :], one1[:, :])
            for d in range(NB):
                nc.gpsimd.dma_start(
                    gsb[d][:, :, :], gx[:, d * KT * 1024 : (d + 1) * KT * 1024]
                )

            for d in range(NB):
                cp = (
                    psum.tile([128, 1024], F32, tag="sim", name=f"cp{d}")
                    if 1 <= d <= NCS
                    else None
                )
                for mt in range(RT):
                    ms = slice(mt * 128, (mt + 1) * 128)
                    sp = psum.tile([128, 1024], F32, tag="sim")
                    for h in range(2):
                        for kp in range(KT // 2):
                            nc.tensor.matmul(
                                sp[:, h * 512 : (h + 1) * 512],
                                gsb[0][:, 2 * kp : 2 * kp + 2, ms],
                                gsb[d][:, 2 * kp : 2 * kp + 2, h * 512 : (h + 1) * 512],
                                start=(kp == 0),
                                stop=(kp == KT // 2 - 1),
                                perf_mode=DR,
                            )
                    if d == 0:
                        h = mt // 4
                        woff = 896 - 128 * mt + 512 * h
                        nc.vector.tensor_add(
                            sp[:, h * 512 : (h + 1) * 512],
                            sp[:, h * 512 : (h + 1) * 512],
                            wsb[:, woff : woff + 512],
                        )
                    if 1 <= d <= NCS:
                        ex = work.tile([128, 1024], BF16, tag="ex")
                        nc.scalar.activation(
                            ex[:, :], sp[:, :], AF.Exp,
                            scale=1.0 / (SC * SC),
                            accum_out=acc[:, mt, d : d + 1],
                        )
                        # exp column-sums, accumulated in PSUM across the mt loop
                        for h in range(2):
                            nc.tensor.matmul(
                                cp[0:1, h * 512 : (h + 1) * 512],
                                osb[:, :],
                                ex[:, h * 512 : (h + 1) * 512],
                                start=(mt == 0),
                                stop=(mt == RT - 1),
                            )
                    else:
                        nc.scalar.activation(
                            sp[:, :], sp[:, :], AF.Exp,
                            scale=1.0 / (SC * SC),
                            accum_out=acc[:, mt, d : d + 1],
                        )
                if 1 <= d <= NCS:
                    off = (d - 1) * 1024
                    nc.vector.tensor_copy(cs[:, off : off + 1024], cp[0:1, :])

            for mt in range(RT):
                nc.vector.reduce_sum(den[:, mt : mt + 1], acc[:, mt, :], axis=AX)

            # P^T = C^T @ G over own rows (C stationary, fp8)
            pt = psum.tile([128, 1024], F32, tag="sim", name="pt")
            for h in range(2):
                for kt in range(KT):
                    nc.tensor.matmul(
                        pt[0:NT, h * 512 : (h + 1) * 512],
                        csb[:, kt * NT : (kt + 1) * NT],
                        gsb[0][:, kt, h * 512 : (h + 1) * 512],
                        start=(kt == 0),
                        stop=(kt == KT - 1),
                    )
            nc.vector.tensor_copy(post[:, :], pt[0:NT, :])

            nc.gpsimd.dma_start(deno[:, :], den[:, :])
            nc.gpsimd.dma_start(cso[:, :], cs[:, :])
            nc.gpsimd.dma_start(poso[:, :], post[:, :])

    nc.compile()
    return nc


_NC_CACHE = None
_last_in_maps = None


def _get_program():
    global _NC_CACHE
    if _NC_CACHE is None:
        _NC_CACHE = build_program()
    return _NC_CACHE


def _build_in_maps(f, t):
    """f: [N, D] float32 features, t: [N] int64 types -> per-core input dicts."""
    nrm = np.maximum(np.sqrt((f.astype(np.float64) ** 2).sum(axis=1)), 1e-12)
    g = (f / nrm[:, None].astype(np.float32)) * np.float32(1.0 / np.sqrt(T))
    g8 = (g * np.float32(SC)).astype(F8_NP)            # [N, D] fp8, x16

    # per-block [128, KT*1024]: B[b][p, kt*1024 + j] = g8.T[kt*128+p, b*1024+j]
    gT = np.ascontiguousarray(g8.T)                     # [D, N]
    Gb = gT.reshape(KT, 128, NC, 1024)                  # [kt, p, b, j]
    blocks = [
        np.ascontiguousarray(Gb[:, :, b, :].transpose(1, 0, 2).reshape(128, KT * 1024))
        for b in range(NC)
    ]

    OH = (t[:, None] == np.arange(NT)[None, :]).astype(np.float32)
    C = g.T @ OH                                        # [D, NT] fp32 (class sums)
    c8 = np.ascontiguousarray(
        C.reshape(KT, 128, NT).transpose(1, 0, 2).reshape(128, KT * NT)
    ).astype(F8_NP)

    wmask = np.zeros((128, WWIN), BF16_NP)
    wmask[np.arange(128), 896 + np.arange(128)] = BF16_NP(NEG)
    one1 = np.ones((128, 1), BF16_NP)

    in_maps = []
    for c in range(NC):
        in_maps.append(
            {
                "gx": np.concatenate(
                    [blocks[(c + d) % NC] for d in range(NB)], axis=1
                ),
                "cc8": c8,
                "wm": wmask,
                "one1": one1,
            }
        )
    return in_maps


def kernel(features, element_types):
    f = np.ascontiguousarray(np.asarray(features), dtype=np.float32)
    t = np.asarray(element_types).astype(np.int64)
    assert f.shape == (N, D) and t.shape == (N,)

    hist = np.bincount(t, minlength=NT)
    cnt = hist[t] - 1
    valid = cnt > 0
    in_maps = _build_in_maps(f, t)
    global _last_in_maps
    _last_in_maps = in_maps

    nc = _get_program()
    res = run_bass_kernel_spmd(nc, in_maps, list(range(NC))).results

    # den_total[i] = own row-sums (blocks d=0..4) + column-sums routed from
    # the cores that computed blocks whose columns are rows of core b.
    den_total = np.zeros(N, dtype=np.float64)
    for c in range(NC):
        den_total[c * R : (c + 1) * R] += np.asarray(
            res[c]["den_o"], dtype=np.float64
        ).T.ravel()
        cs = np.asarray(res[c]["cs_o"], dtype=np.float64).ravel()
        for d in range(1, NCS + 1):
            b = (c + d) % NC
            den_total[b * R : (b + 1) * R] += cs[(d - 1) * 1024 : d * 1024]

    total = 0.0
    vc = int(valid.sum())
    for c in range(NC):
        PT = np.asarray(res[c]["pos_o"], dtype=np.float64) / SC   # [NT, R]
        rows = slice(c * R, (c + 1) * R)
        pos_sum = PT[t[rows], np.arange(R)] - 1.0 / T
        pm = pos_sum / np.maximum(cnt[rows], 1)
        loss = -np.log(np.exp(pm) / den_total[rows] + EPS)
        total += float((loss * valid[rows]).sum())
    out = total / vc if vc > 0 else 0.0
    return np.float32(out)


# revision 20
# speedup vs baseline: 1.4375x; 1.3845x over previous
"""Supervised contrastive loss kernel for Trainium2 (8 NeuronCores, Bass/Tile).

Row-parallel sharding with host-side input staging and symmetric halving:
core c owns rows [c*1024, (c+1)*1024).  The host L2-normalizes rows (scale
folded with 1/sqrt(temperature)), transposes to gT [D, N], quantizes to
fp8e4m3 (x16 scaling for range), and hands each core 5 of the 8
1024-column blocks in circulant order (own block first, then the next 4
mod 8) -- exploiting sim-matrix symmetry: block pair (a, b) is computed
once, and its exp column-sums are routed back to the owner of rows b.

Per core (d = block index 0..4):
  - sim chunk [128, 1024] per (d, m-tile) via fp8 DoubleRow matmuls
    (2 fp8 weights per PE cell => K=256 per instruction, fp32 PSUM accum),
  - d=0: additive -1e9 diagonal-window mask (SPMD-uniform thanks to the
    rotation), exp in place on PSUM, row-sums via DVE reduce,
  - d=1..3: exp written to SBUF as fp8 (+ fused ACT row-sum accumulation);
    exp column-sums via DoubleRow ones-vector matmuls accumulated in PSUM
    across the m-tile loop, then copied to a [1, 3072] staging buffer,
  - d=4: exp row-sums only (the transpose pair computes its own row-sums),
  - P^T = C^T @ G (class-sum matrix C stationary, fp8, DoubleRow).
Every matmul in the program is DoubleRow fp8 -- no PE mode switches.
Outputs per core: den [128, 8], colsums [1, 3072], P^T [32, 1024].

Host combines: den_total[row] = own row-sums + 3 routed column-sums;
pos_i = P[i, type_i]/16 - 1/T; then the standard log-form reduction.
"""

import numpy as np
import ml_dtypes

import concourse.bass as bass
import concourse.bacc as bacc
import concourse.mybir as mybir
from concourse import tile
from concourse.bass_utils import run_bass_kernel_spmd

N, D, NT, NC = 8192, 1024, 32, 8
R = N // NC          # rows per core
RT = R // 128        # row tiles per core (m-tiles)
KT = D // 128        # contraction chunks
NB = 5               # column blocks computed per core (symmetric halving)
NCS = 3              # blocks with column-sum routing (d = 1..3)
T = 0.07
EPS = 1e-10
NEG = -1.0e9
SC = 16.0            # fp8 pre-scale per operand (PSUM carries SC^2 * sim)
WWIN = 896           # width of the compact shifted diagonal-mask input

F32 = mybir.dt.float32
BF16 = mybir.dt.bfloat16
F8 = mybir.dt.float8e4
BF16_NP = ml_dtypes.bfloat16
F8_NP = ml_dtypes.float8_e4m3


def build_program():
    nc = bacc.Bacc(None, target_bir_lowering=False, debug=False)
    gx = nc.dram_tensor("gx", [128, NB * KT * 1024], F8, kind="ExternalInput")
    cc8 = nc.dram_tensor("cc8", [128, KT * NT], F8, kind="ExternalInput")
    wm = nc.dram_tensor("wm", [128, WWIN], BF16, kind="ExternalInput")
    one1 = nc.dram_tensor("one1", [128, 32], F8, kind="ExternalInput")
    deno = nc.dram_tensor("den_o", [128, RT], F32, kind="ExternalOutput")
    cso = nc.dram_tensor("cs_o", [1, NCS * 1024], F32, kind="ExternalOutput")
    poso = nc.dram_tensor("pos_o", [NT, R], F32, kind="ExternalOutput")

    AX = mybir.AxisListType.X
    AF = mybir.ActivationFunctionType
    DR = mybir.MatmulPerfMode.DoubleRow

    with tile.TileContext(nc) as tc:
        with (
            tc.tile_pool(name="big", bufs=1) as big,
            tc.tile_pool(name="work", bufs=2) as work,
            tc.tile_pool(name="stats", bufs=1) as stats,
            tc.tile_pool(name="psum", bufs=4, space="PSUM") as psum,
        ):
            gsb = [
                big.tile([128, KT, 1024], F8, tag=f"g{d}", name=f"g{d}")
                for d in range(NB)
            ]
            csb = big.tile([128, KT, NT], F8, tag="cc")
            wsb = big.tile([128, WWIN], BF16, tag="wm")
            osb = big.tile([128, 2, 16], F8, tag="one")
            acc = stats.tile([128, RT, NB], F32, tag="acc")
            den = stats.tile([128, RT], F32, tag="den")
            cs = stats.tile([1, NCS * 1024], F32, tag="cs")
            post = stats.tile([NT, R], F32, tag="post")

            # own block first (kt-pair granularity so the PE starts early),
            # then the mask + ones (needed early), then the remaining blocks,
            # then the class sums (needed only by the trailing P matmuls).
            for k in range(KT // 2):
                nc.gpsimd.dma_start(
                    gsb[0][:, 2 * k : 2 * k + 2, :],
                    gx[:, 2 * k * 1024 : (2 * k + 2) * 1024],
                )
            nc.gpsimd.dma_start(wsb[:, :], wm[:, :])
            nc.gpsimd.dma_start(osb[:, :, :], one1[:, :])
            for d in range(1, NB):
                nc.gpsimd.dma_start(
                    gsb[d][:, :, :], gx[:, d * KT * 1024 : (d + 1) * KT * 1024]
                )
            nc.gpsimd.dma_start(csb[:, :, :], cc8[:, :])

            for d in range(NB):
                cp = (
                    psum.tile([128, 1024], F32, tag="sim", name=f"cp{d}")
                    if 1 <= d <= NCS
                    else None
                )
                ex = (
                    work.tile([128, RT, 1024], F8, tag="ex", name=f"ex{d}")
                    if 1 <= d <= NCS
                    else None
                )
                for mt in range(RT):
                    ms = slice(mt * 128, (mt + 1) * 128)
                    sp = psum.tile([128, 1024], F32, tag="sim")
                    for h in range(2):
                        for kp in range(KT // 2):
                            nc.tensor.matmul(
                                sp[:, h * 512 : (h + 1) * 512],
                                gsb[0][:, 2 * kp : 2 * kp + 2, ms],
                                gsb[d][:, 2 * kp : 2 * kp + 2, h * 512 : (h + 1) * 512],
                                start=(kp == 0),
                                stop=(kp == KT // 2 - 1),
                                perf_mode=DR,
                            )
                    if d == 0:
                        h = mt // 4
                        woff = 384 - 128 * (mt % 4)
                        nc.vector.tensor_add(
                            sp[:, h * 512 : (h + 1) * 512],
                            sp[:, h * 512 : (h + 1) * 512],
                            wsb[:, woff : woff + 512],
                        )
                    if 1 <= d <= NCS:
                        nc.scalar.activation(
                            ex[:, mt, :], sp[:, :], AF.Exp,
                            scale=1.0 / (SC * SC),
                            accum_out=acc[:, mt, d : d + 1],
                        )
                    else:
                        nc.scalar.activation(
                            sp[:, :], sp[:, :], AF.Exp, scale=1.0 / (SC * SC)
                        )
                        nc.vector.reduce_sum(
                            acc[:, mt, d : d + 1], sp[:, :], axis=AX
                        )
                if 1 <= d <= NCS:
                    # exp column-sums: DoubleRow ones-matmuls over mt pairs,
                    # accumulated in PSUM
                    for h in range(2):
                        for mp in range(RT // 2):
                            nc.tensor.matmul(
                                cp[0:1, h * 512 : (h + 1) * 512],
                                osb[:, :, 0:1],
                                ex[:, 2 * mp : 2 * mp + 2, h * 512 : (h + 1) * 512],
                                start=(mp == 0),
                                stop=(mp == RT // 2 - 1),
                                perf_mode=DR,
                            )
                    off = (d - 1) * 1024
                    nc.vector.tensor_copy(cs[:, off : off + 1024], cp[0:1, :])

            for mt in range(RT):
                nc.vector.reduce_sum(den[:, mt : mt + 1], acc[:, mt, :], axis=AX)

            # P^T = C^T @ G over own rows (C stationary, fp8, DoubleRow)
            pt = psum.tile([128, 1024], F32, tag="sim", name="pt")
            for h in range(2):
                for kp in range(KT // 2):
                    nc.tensor.matmul(
                        pt[0:NT, h * 512 : (h + 1) * 512],
                        csb[:, 2 * kp : 2 * kp + 2, :],
                        gsb[0][:, 2 * kp : 2 * kp + 2, h * 512 : (h + 1) * 512],
                        start=(kp == 0),
                        stop=(kp == KT // 2 - 1),
                        perf_mode=DR,
                    )
            nc.vector.tensor_copy(post[:, :], pt[0:NT, :])

            nc.gpsimd.dma_start(deno[:, :], den[:, :])
            nc.gpsimd.dma_start(cso[:, :], cs[:, :])
            nc.gpsimd.dma_start(poso[:, :], post[:, :])

    nc.compile()
    return nc


_NC_CACHE = None
_last_in_maps = None


def _get_program():
    global _NC_CACHE
    if _NC_CACHE is None:
        _NC_CACHE = build_program()
    return _NC_CACHE


def _build_in_maps(f, t):
    """f: [N, D] float32 features, t: [N] int64 types -> per-core input dicts."""
    nrm = np.maximum(np.sqrt((f.astype(np.float64) ** 2).sum(axis=1)), 1e-12)
    g = (f / nrm[:, None].astype(np.float32)) * np.float32(1.0 / np.sqrt(T))
    g8 = (g * np.float32(SC)).astype(F8_NP)            # [N, D] fp8, x16

    # per-block [128, KT*1024]: B[b][p, kt*1024 + j] = g8.T[kt*128+p, b*1024+j]
    gT = np.ascontiguousarray(g8.T)                     # [D, N]
    Gb = gT.reshape(KT, 128, NC, 1024)                  # [kt, p, b, j]
    blocks = [
        np.ascontiguousarray(Gb[:, :, b, :].transpose(1, 0, 2).reshape(128, KT * 1024))
        for b in range(NC)
    ]

    OH = (t[:, None] == np.arange(NT)[None, :]).astype(np.float32)
    C = g.T @ OH                                        # [D, NT] fp32 (class sums)
    c8 = np.ascontiguousarray(
        C.reshape(KT, 128, NT).transpose(1, 0, 2).reshape(128, KT * NT)
    ).astype(F8_NP)

    wmask = np.zeros((128, WWIN), BF16_NP)
    wmask[np.arange(128), 384 + np.arange(128)] = BF16_NP(NEG)
    one1 = np.ones((128, 32), F8_NP)

    in_maps = []
    for c in range(NC):
        in_maps.append(
            {
                "gx": np.concatenate(
                    [blocks[(c + d) % NC] for d in range(NB)], axis=1
                ),
                "cc8": c8,
                "wm": wmask,
                "one1": one1,
            }
        )
    return in_maps


def kernel(features, element_types):
    f = np.ascontiguousarray(np.asarray(features), dtype=np.float32)
    t = np.asarray(element_types).astype(np.int64)
    assert f.shape == (N, D) and t.shape == (N,)

    hist = np.bincount(t, minlength=NT)
    cnt = hist[t] - 1
    valid = cnt > 0
    in_maps = _build_in_maps(f, t)
    global _last_in_maps
    _last_in_maps = in_maps

    nc = _get_program()
    res = run_bass_kernel_spmd(nc, in_maps, list(range(NC))).results

    # den_total[i] = own row-sums (blocks d=0..4) + column-sums routed from
    # the cores that computed blocks whose columns are rows of core b.
    den_total = np.zeros(N, dtype=np.float64)
    for c in range(NC):
        den_total[c * R : (c + 1) * R] += np.asarray(
            res[c]["den_o"], dtype=np.float64
        ).T.ravel()
        cs = np.asarray(res[c]["cs_o"], dtype=np.float64).ravel()
        for d in range(1, NCS + 1):
            b = (c + d) % NC
            den_total[b * R : (b + 1) * R] += cs[(d - 1) * 1024 : d * 1024]

    total = 0.0
    vc = int(valid.sum())
    for c in range(NC):
        PT = np.asarray(res[c]["pos_o"], dtype=np.float64) / SC   # [NT, R]
        rows = slice(c * R, (c + 1) * R)
        pos_sum = PT[t[rows], np.arange(R)] - 1.0 / T
        pm = pos_sum / np.maximum(cnt[rows], 1)
        loss = -np.log(np.exp(pm) / den_total[rows] + EPS)
        total += float((loss * valid[rows]).sum())
    out = total / vc if vc > 0 else 0.0
    return np.float32(out)
